# revision 1
# baseline (speedup 1.0000x reference)
"""Bezier-to-image Gaussian splat kernel for Trainium2 (8 NeuronCores).

Reference computation (per sample b of 256):
    T = warped cubic Bernstein basis (30, 4)
    points = einsum('nk,blkc->blnc', T, x.reshape(B,160,4,2))   # (B,160,30,2)
    gx[b,l,i,n] = exp(-(i/60 - X[b,l,n])^2 / 2e-4)
    out[b,i,j]  = min(sum_{l,n} gx[b,l,i,n]*gy[b,l,j,n], 1)     # (B,60,60)

Strategy: pure data parallel, 32 samples per core.  Per sample the 4800
bezier points are processed in 40 chunks of 128 points (4 curves x 32-row
strips; rows 30/31 of each strip are dead and killed via the iota constant);
d[p,i] = i - 60*X_p is built by one broadcast DVE tensor_tensor reading r
straight from PSUM, the Gaussian is evaluated on ScalarE (Derivative_Erf
LUT = 2/sqrt(pi)*exp(-x^2) in a single batched pass), and the 60x60 image
accumulates on PE as sum_c GxT_c^T @ GyT_c in one PSUM bank.

Pipelining: the r matmuls are emitted two samples ahead so the DVE subtract
never waits on PE, and the (PSUM-dependent) min/scale op for sample b is
emitted one iteration late so its wait on the image matmuls overlaps the
next sample's subtract instead of stalling the in-order DVE queue.
"""

import math

import numpy as np
import orjson

import bass_rust
import concourse.bass as bass
import concourse.mybir as mybir
import concourse.tile as tile
from concourse.bass_utils import run_bass_kernel_spmd

B, L, N, W = 256, 160, 30, 60
NCORES = 8
BC = B // NCORES          # samples per core
ALPHA = 2e-4
KEXP = 1.0 / (W * W * ALPHA)          # exponent scale in cell units: 1/0.72
SDERF = math.sqrt(KEXP)               # Derivative_Erf input scale
DERF_FIX = math.pi / 4.0              # undo (2/sqrt(pi))^2 from Derivative_Erf
CHUNKS = 40                           # 4 curves x 30 samples per chunk
PTS = 128                             # chunk partition dim: p = 32*lg + n
CW = 60                               # width of one chunk's band (= W)

LAST_RESULTS = None  # test harness reads profiling info from here


def _basis_T() -> np.ndarray:
    t = np.arange(N, dtype=np.float32) / np.float32(N)
    t = 2 * t**3 - 3 * t**2 + 2 * t
    t_3_0 = t**3
    t_2_1 = t**2 - t_3_0
    t_1_2 = t_3_0 - 2 * t**2 + t
    t_0_3 = (1 - t) ** 3
    return np.stack([t_3_0, 3 * t_2_1, 3 * t_1_2, t_0_3], axis=1).astype(np.float32)


def _legalize_waits(nc, max_waits: int = 1):
    """Walrus rejects engine instructions carrying more than ~1 sync wait
    ("Too many sync wait commands").  Hoist excess waits onto same-engine
    Drain instructions inserted immediately before the offender."""
    js = orjson.loads(mybir.module_to_json_bytes(nc.m))
    ctr = 0
    for f in js["functions"]:
        for bb in f["blocks"]:
            out = []
            changed = False
            for inst in bb["instructions"]:
                si = inst.get("sync_info")
                waits = si.get("on_wait") if si else None
                if waits and len(waits) > max_waits:
                    keep = waits[:max_waits]
                    for w in waits[max_waits:]:
                        ctr += 1
                        out.append({
                            "debug": inst.get("debug", 0),
                            "engine": inst["engine"],
                            "ins": [], "outs": [],
                            "name": f"waitfix-{ctr}",
                            "opcode": "Drain",
                            "sync_info": {"on_update": [], "on_wait": [w]},
                        })
                    si["on_wait"] = keep
                    changed = True
                out.append(inst)
            if changed:
                bb["instructions"] = out
    if ctr:
        nc.m = bass_rust.module_from_json_bytes(orjson.dumps(js))
    return ctr


def build_program(legalize: bool = True):
    f32 = mybir.dt.float32
    f16 = mybir.dt.float16

    nc = bass.Bass("TRN2", target_bir_lowering=False, debug=False)

    x_t = nc.dram_tensor("x", [BC, L, 8], f32, kind="ExternalInput")
    y_t = nc.dram_tensor("y", [BC, W, W], f32, kind="ExternalOutput")

    # (4, 32) stationary operand: r[m] = sum_k TscT[k,m]*ctrl[k] = 60*X.
    tsc_np = np.zeros((4, 32), dtype=np.float32)
    tsc_np[:, :N] = (W * _basis_T()).T
    tsc_d = nc.inline_tensor(tsc_np, name="tscT")

    # x-side iota (chunk-major band layout): dead rows (n in {30,31} of each
    # 32-strip) get +120 so their distance is >= 60 -> gx = 0, killing the
    # dead rows' contribution to the outer product regardless of the y side.
    iota_np = np.tile(np.arange(CW, dtype=np.float16), (PTS, 1))
    for lg in range(4):
        iota_np[32 * lg + 30 : 32 * lg + 32, :] += np.float16(120.0)
    iota_d = nc.inline_tensor(iota_np, name="iota60")


    with tile.TileContext(nc) as tc, tc.tile_pool(name="const", bufs=1) as cpool, \
            tc.tile_pool(name="ctrl", bufs=1) as ctrl_pool, \
            tc.tile_pool(name="outp", bufs=1) as out_pool, \
            tc.tile_pool(name="stage", bufs=1) as stage_pool, \
            tc.tile_pool(name="dwork", bufs=3) as dpool, \
            tc.tile_pool(name="band", bufs=4) as band_pool, \
            tc.tile_pool(name="rpsum", bufs=3, space="PSUM") as rps_pool, \
            tc.tile_pool(name="imgpsum", bufs=3, space="PSUM") as img_pool:

        # Prologue: DMA loads land in staging tiles; DVE copies them into the
        # tiles PE reads (PE LDWEIGHTS tolerates very few sync waits).
        tsc0 = cpool.tile([4, 32], f32, tag="tsc0")
        nc.sync.dma_start(tsc0[:], tsc_d.ap())
        tsc = cpool.tile([4, 32], f32, tag="tsc")
        nc.vector.tensor_copy(tsc[:], tsc0[:])
        # iota rides the Activation DGE queue: its ~128 descriptors would
        # otherwise delay the critical first ct stage on the SP queue.
        iot = cpool.tile([PTS, CW], f16, tag="iota")
        nc.scalar.dma_start(iot[:], iota_d.ap())

        # control points: partition k (4), free = (b, l, coord), loaded in
        # stages.  The PE reads the staging tiles directly (the matmul just
        # carries one DMA-queue wait), and stages alternate between the SP
        # and Activation DGE queues so neighbouring stages transfer in
        # parallel.  Descriptor processing is the hidden long pole: ~21k
        # 8-byte-run descriptors trickle for most of the kernel.
        GRP = 8
        stages = [(0, 1), (1, 2), (2, 5), (5, 8), (8, 16), (16, 24), (24, 32)]
        # queue plan: tiny stages 0-1 sequential on the SP queue, stage 2
        # leads the Activation queue so sample 2 is never DMA-gated, big
        # stages land where cumulative descriptor time leaves slack.
        stage_eng = ["sync", "sync", "scalar", "scalar", "sync", "scalar", "sync"]
        ct_view = []
        stage_of = {}
        for si, (b0, b1) in enumerate(stages):
            ct0 = stage_pool.tile([4, (b1 - b0) * 2 * L], f32, tag=f"ct{si}")
            eng = nc.sync if stage_eng[si] == "sync" else nc.scalar
            eng.dma_start(
                ct0[:].rearrange("k (b l c) -> k b l c", b=b1 - b0, c=2),
                x_t.ap()[b0:b1].rearrange("b l (k c) -> k b l c", k=4),
            )
            ct_view.append(
                ct0[:].rearrange(
                    "k (b c g co) -> k b c g co", b=b1 - b0, c=CHUNKS, co=2
                )
            )
            for b in range(b0, b1):
                stage_of[b] = si

        # all 32 output images live here until the per-group DMAs
        out_all = out_pool.tile([W, BC * W], f32, tag="oall")

        CS_ALL = 2 * CHUNKS
        r_tiles = {}
        img_tiles = {}

        def emit_r(b, nb, tag):
            """r matmuls for samples [b, b+nb) into one PSUM tile."""
            si = stage_of[b]
            assert stage_of[b + nb - 1] == si
            bl = b - stages[si][0]
            r_ps = rps_pool.tile([PTS, nb * 2 * CHUNKS], f32, tag=tag)
            for lg in range(4):
                nc.tensor.matmul(
                    r_ps[32 * lg : 32 * lg + 32, :],
                    lhsT=tsc[:],
                    rhs=ct_view[si][:, bl : bl + nb, :, lg : lg + 1, :],
                    start=True,
                    stop=True,
                    tile_position=(0, 32 * lg),
                )
            r_tiles[b] = r_ps

        def emit_min(b):
            """min(s*img, 1) = 1 - relu(1 - s*img), on ScalarE (which has
            slack; keeping this off the in-order DVE queue avoids stalling
            the next subtract on the image matmuls).  Group DMA when a
            group closes."""
            img = img_tiles.pop(b)
            tmp = dpool.tile([W, W], f32, tag="mintmp")
            nc.scalar.activation(
                tmp[:], img[:],
                mybir.ActivationFunctionType.Relu,
                bias=1.0, scale=-DERF_FIX,
            )
            nc.scalar.activation(
                out_all[:, W * b : W * (b + 1)], tmp[:],
                mybir.ActivationFunctionType.Copy,
                bias=1.0, scale=-1.0,
            )
            if b >= BC - GRP:
                # last group: per-sample DMAs so the final transfer is tiny
                nc.sync.dma_start(
                    y_t.ap()[b : b + 1].rearrange("b i j -> i b j"),
                    out_all[:, W * b : W * (b + 1)]
                    .rearrange("i (b j) -> i b j", b=1),
                )
            elif b % GRP == GRP - 1:
                g = b // GRP
                nc.sync.dma_start(
                    y_t.ap()[g * GRP : (g + 1) * GRP].rearrange("b i j -> i b j"),
                    out_all[:, W * GRP * g : W * GRP * (g + 1)]
                    .rearrange("i (b j) -> i b j", b=GRP),
                )

        SAMP = 2 * CHUNKS * CW          # band elems per sample

        def emit_band(r_ps, nb, dd, gg, off, act_halves=1):
            """subtract + Gaussian for nb samples into dd/gg at offset.
            (nb=1 keeps 3-dim APs: a singleton 4th dim costs ~20% DVE.)"""
            if nb == 1:
                nc.vector.tensor_tensor(
                    dd[:, off : off + SAMP].rearrange("p (cs w) -> p cs w", w=CW),
                    iot[:].rearrange("p (o w) -> p o w", o=1).broadcast_to(
                        [PTS, CS_ALL, CW]
                    ),
                    r_ps[:].rearrange("p (cs o) -> p cs o", o=1).broadcast_to(
                        [PTS, CS_ALL, CW]
                    ),
                    mybir.AluOpType.subtract,
                )
            else:
                nc.vector.tensor_tensor(
                    dd[:, off : off + nb * SAMP].rearrange(
                        "p (b2 cs w) -> p b2 cs w", b2=nb, w=CW
                    ),
                    iot[:].rearrange("p (o q w) -> p o q w", o=1, q=1)
                    .broadcast_to([PTS, nb, CS_ALL, CW]),
                    r_ps[:].rearrange("p (b2 cs o) -> p b2 cs o", b2=nb, o=1)
                    .broadcast_to([PTS, nb, CS_ALL, CW]),
                    mybir.AluOpType.subtract,
                )
            n = nb * SAMP
            h = n // act_halves
            for k in range(act_halves):
                nc.scalar.activation(
                    gg[:, off + k * h : off + (k + 1) * h],
                    dd[:, off + k * h : off + (k + 1) * h],
                    mybir.ActivationFunctionType.Derivative_Erf,
                    bias=0.0, scale=SDERF,
                )

        def emit_img(gg, off, b, c0=0, c1=CHUNKS):
            if b in img_tiles:
                img = img_tiles[b]
            else:
                img = img_pool.tile([W, W], f32, tag="img")
                img_tiles[b] = img
            for c in range(c0, c1):
                nc.tensor.matmul(
                    img[:],
                    lhsT=gg[:, off + 2 * CW * c : off + 2 * CW * c + W],
                    rhs=gg[:, off + 2 * CW * c + CW : off + 2 * CW * c + CW + W],
                    start=(c == 0),
                    stop=(c == CHUNKS - 1),
                )

        # software pipeline: r two samples ahead, min one sample behind.
        emit_r(0, 1, "rps")
        emit_r(1, 1, "rps")

        for b in range(BC):
            if b + 2 < BC:
                emit_r(b + 2, 1, "rps")
            r_ps = r_tiles.pop(b)
            dd = band_pool.tile([PTS, SAMP], f16, tag="dd")
            gg = band_pool.tile([PTS, SAMP], f16, tag="gg")
            emit_band(r_ps, 1, dd, gg, 0,
                      act_halves=2 if b in (0, BC - 1) else 1)
            emit_img(gg, 0, b)
            if b > 0:
                emit_min(b - 1)
        emit_min(BC - 1)

    if legalize:
        _legalize_waits(nc)
    return nc


_PROGRAM = None


def kernel(x: np.ndarray, _trace: bool = False) -> np.ndarray:
    global _PROGRAM, LAST_RESULTS
    assert x.shape == (B, L, 8) and x.dtype == np.float32, (x.shape, x.dtype)
    if _PROGRAM is None:
        _PROGRAM = build_program()
    nc = _PROGRAM
    shards = np.split(np.ascontiguousarray(x), NCORES, axis=0)
    in_maps = [{"x": s} for s in shards]
    res = run_bass_kernel_spmd(nc, in_maps, list(range(NCORES)), trace=_trace)
    LAST_RESULTS = res
    return np.concatenate([res.results[i]["y"] for i in range(NCORES)], axis=0)

